# revision 1
# baseline (speedup 1.0000x reference)
"""Trainium2 Bass kernel for nn_AttentionBlock (B=4, C=128, T=4096, K=64, V=128).

Sharding: 8 cores = 4 batches x 2 j-groups (pure data parallel over batch,
plus a split of the key/value axis j). The causal structure (only i <= j
survives the mask, softmax runs over the query axis i which is local to a
j-column) makes a j-split embarrassingly parallel up to a final sum of
partial read matrices, which the host performs.

j-tiles (128 wide) are interleaved between the two j-group cores so the
triangular live region is load-balanced and, crucially, the number of live
512-wide i-chunks per local j-tile index is IDENTICAL on every core
(d_k = k//2 + 1), so one SPMD program serves all 8 cores; only input data
(x slice, gathered j-columns, additive mask tiles) differs per core.

Device computes outT = read^T partial [V=128, T] directly (the output needs
[B, C+V, T]; rows C: are read^T and rows :C are x itself, so the x
passthrough costs nothing on device).
"""

import numpy as np

_B, _C, _T = 4, 128, 4096
_K, _V = 64, 128
_JT = 16          # local 128-wide j tiles per core -> 2048 local j columns
_CH = 512         # i-chunk width (one PSUM bank in fp32)
_ICH = _T // _CH  # 8 i-chunks

_NEG = -1.0e30    # effective -inf for the causal mask (exp -> 0 exactly)

_cache = {}


def _build_nc():
    from contextlib import ExitStack

    import concourse.tile as tile
    from concourse import bacc, mybir
    from concourse.masks import make_identity

    f32 = mybir.dt.float32
    bf16 = mybir.dt.bfloat16
    AF = mybir.ActivationFunctionType

    nc = bacc.Bacc("TRN2", target_bir_lowering=False)

    xb_d = nc.dram_tensor("xb", [_C, _T], f32, kind="ExternalInput")
    xj_d = nc.dram_tensor("xj", [_C, _JT * 128], f32, kind="ExternalInput")
    wq_d = nc.dram_tensor("wq", [_C, _K], f32, kind="ExternalInput")
    wk_d = nc.dram_tensor("wk", [_C, _K], f32, kind="ExternalInput")
    wv_d = nc.dram_tensor("wv", [_C, _V], f32, kind="ExternalInput")
    bq_d = nc.dram_tensor("bq", [_K, 1], f32, kind="ExternalInput")
    bk_d = nc.dram_tensor("bk", [_K, 1], f32, kind="ExternalInput")
    bv_d = nc.dram_tensor("bv", [1, _V], f32, kind="ExternalInput")
    mk_d = nc.dram_tensor("mask", [2, 128, _CH], f32, kind="ExternalInput")
    out_d = nc.dram_tensor("out", [_V, _T], f32, kind="ExternalOutput")

    with tile.TileContext(nc) as tc, ExitStack() as ctx:
        singles = ctx.enter_context(tc.tile_pool(name="singles", bufs=1))
        work = ctx.enter_context(tc.tile_pool(name="work", bufs=3))
        small = ctx.enter_context(tc.tile_pool(name="small", bufs=4))
        psum = ctx.enter_context(tc.tile_pool(name="psum", bufs=2, space="PSUM"))

        # ---------------- load + cast to bf16 ----------------
        xb_bf = singles.tile([_C, _T], bf16)
        xj_bf = singles.tile([_C, _JT * 128], bf16)
        for c in range(_ICH):
            t = work.tile([_C, _CH], f32, tag="ld")
            nc.sync.dma_start(out=t, in_=xb_d[:, c * _CH:(c + 1) * _CH])
            nc.vector.tensor_copy(xb_bf[:, c * _CH:(c + 1) * _CH], t)
        for c in range(_JT * 128 // _CH):
            t = work.tile([_C, _CH], f32, tag="ld")
            nc.sync.dma_start(out=t, in_=xj_d[:, c * _CH:(c + 1) * _CH])
            nc.vector.tensor_copy(xj_bf[:, c * _CH:(c + 1) * _CH], t)

        wq_bf = singles.tile([_C, _K], bf16)
        wk_bf = singles.tile([_C, _K], bf16)
        wv_bf = singles.tile([_C, _V], bf16)
        for d_, t_ in ((wq_d, wq_bf), (wk_d, wk_bf), (wv_d, wv_bf)):
            w = d_.shape[1]
            tmp = work.tile([_C, _V], f32, tag="wld")
            nc.sync.dma_start(out=tmp[:, :w], in_=d_[:])
            nc.vector.tensor_copy(t_, tmp[:, :w])

        bq_s = singles.tile([_K, 1], f32)
        nc.sync.dma_start(out=bq_s, in_=bq_d[:])
        bk_s = singles.tile([_K, 1], f32)
        nc.sync.dma_start(out=bk_s, in_=bk_d[:])
        bv_s = singles.tile([1, _V], f32)
        nc.sync.dma_start(out=bv_s, in_=bv_d[:])
        bv_bf = singles.tile([1, _V], bf16)
        nc.vector.tensor_copy(bv_bf, bv_s)
        ones_bf = singles.tile([1, 128], bf16)
        nc.vector.memset(ones_bf, 1.0)

        mask_f = singles.tile([128, 2, _CH], f32)
        mask_bf = singles.tile([128, 2, _CH], bf16)
        for m in range(2):
            nc.sync.dma_start(out=mask_f[:, m, :], in_=mk_d[m])
        nc.vector.tensor_copy(mask_bf, mask_f)

        id_bf = singles.tile([128, 128], bf16)
        make_identity(nc, id_bf[:])

        # ---------------- projections ----------------
        # qt[kk, i] = sum_c Wq[c, kk] * x[c, i] + bq[kk]   (Q^T, [64, T])
        qt_bf = singles.tile([_K, _T], bf16)
        for c in range(_ICH):
            ps = psum.tile([128, 2048], f32, tag="ps")
            nc.tensor.matmul(ps[0:_K, 0:_CH], wq_bf,
                             xb_bf[:, c * _CH:(c + 1) * _CH],
                             start=True, stop=True)
            nc.vector.tensor_scalar_add(qt_bf[:, c * _CH:(c + 1) * _CH],
                                        ps[0:_K, 0:_CH], bq_s[:])
        # kt[kk, jl] over this core's gathered j columns ([64, 2048])
        kt_bf = singles.tile([_K, _JT * 128], bf16)
        for c in range(_JT * 128 // _CH):
            ps = psum.tile([128, 2048], f32, tag="ps")
            nc.tensor.matmul(ps[0:_K, 0:_CH], wk_bf,
                             xj_bf[:, c * _CH:(c + 1) * _CH],
                             start=True, stop=True)
            nc.vector.tensor_scalar_add(kt_bf[:, c * _CH:(c + 1) * _CH],
                                        ps[0:_K, 0:_CH], bk_s[:])
        # v[jl, v] = sum_c x[c, jl] * Wv[c, v] + bv[v]   ([128, V] per j-tile)
        v_f32 = singles.tile([128, _JT, _V], f32)
        for k in range(_JT):
            ps = psum.tile([128, 2048], f32, tag="ps")
            nc.tensor.matmul(ps[:, 0:_V], xj_bf[:, k * 128:(k + 1) * 128],
                             wv_bf, start=True, stop=False)
            nc.tensor.matmul(ps[:, 0:_V], ones_bf, bv_bf,
                             start=False, stop=True)
            nc.vector.tensor_copy(v_f32[:, k, :], ps[:, 0:_V])

        # ---------------- attention ----------------
        # Per local j-tile k (descending so read chunks unlock early):
        #   logits^T[jl, i] for live i-chunks only (d = k//2 + 1 of them),
        #   additive causal mask on the diagonal chunk (via PE identity
        #   matmul into the same PSUM accumulation group),
        #   e = exp(logits/8) via ScalarE with row-sum accum,
        #   vs[jl, :] = (V[jl, :]) / s[jl]  in bf16.
        # After tile k (k even): read i-chunk c = k//2 is fully determined:
        #   outT[v, i] += sum_jl vs[jl, v] * e[jl, i] over j-tiles >= 2c.
        e_all = singles.tile([128, _JT, _T], bf16)
        vs_bf = singles.tile([128, _JT, _V], bf16)

        def emit_read_chunk(c):
            ks = list(range(2 * c, _JT))
            ps = psum.tile([128, 2048], f32, tag="ps")
            for i, k in enumerate(ks):
                nc.tensor.matmul(ps[0:_V, 0:_CH], vs_bf[:, k, :],
                                 e_all[:, k, c * _CH:(c + 1) * _CH],
                                 start=(i == 0), stop=(i == len(ks) - 1))
            ot = work.tile([_V, _CH], f32, tag="osb")
            nc.vector.tensor_copy(ot, ps[0:_V, 0:_CH])
            nc.sync.dma_start(out=out_d[:, c * _CH:(c + 1) * _CH], in_=ot)

        for k in range(_JT - 1, -1, -1):
            d = k // 2 + 1
            accs = []
            for g0 in range(0, d, 4):
                g1 = min(g0 + 4, d)
                ps = psum.tile([128, 2048], f32, tag="ps")
                for c in range(g0, g1):
                    diag = (c == d - 1)
                    nc.tensor.matmul(ps[:, (c - g0) * _CH:(c - g0 + 1) * _CH],
                                     kt_bf[:, k * 128:(k + 1) * 128],
                                     qt_bf[:, c * _CH:(c + 1) * _CH],
                                     start=True, stop=not diag)
                    if diag:
                        nc.tensor.matmul(
                            ps[:, (c - g0) * _CH:(c - g0 + 1) * _CH],
                            id_bf, mask_bf[:, k % 2, :],
                            start=False, stop=True)
                acc = small.tile([128, 1], f32, tag="acc")
                nc.scalar.activation(out=e_all[:, k, g0 * _CH:g1 * _CH],
                                     in_=ps[:, 0:(g1 - g0) * _CH],
                                     func=AF.Exp, scale=0.125, accum_out=acc)
                accs.append(acc)
            if len(accs) == 1:
                s_t = accs[0]
            else:
                s_t = small.tile([128, 1], f32, tag="s")
                nc.vector.tensor_add(s_t, accs[0], accs[1])
            rs = small.tile([128, 1], f32, tag="rs")
            nc.vector.reciprocal(rs, s_t)
            nc.vector.tensor_scalar_mul(vs_bf[:, k, :], v_f32[:, k, :], rs)

            if k % 2 == 0:
                emit_read_chunk(k // 2)

    nc.compile()
    return nc


def _get_nc():
    if "nc" not in _cache:
        _cache["nc"] = _build_nc()
    return _cache["nc"]


def _masks(g):
    """Additive causal-mask tiles for a core in j-group g.

    Tile m (= local j-tile parity) masks the diagonal 512-wide i-chunk of
    every local j-tile with that parity: entry [p, ii] is live iff
    global_i <= global_j, i.e. ii <= (j0 - i0) + p with j0 - i0 = 128*g + 256*m.
    """
    m = np.zeros((2, 128, _CH), np.float32)
    p = np.arange(128)[:, None]
    ii = np.arange(_CH)[None, :]
    for parity in range(2):
        o = 128 * g + 256 * parity
        m[parity] = np.where(ii <= o + p, 0.0, _NEG)
    return m


def kernel(**inputs):
    from concourse.bass_utils import run_bass_kernel_spmd

    x = np.ascontiguousarray(np.asarray(inputs["x"], dtype=np.float32))
    Wq = np.ascontiguousarray(np.asarray(inputs["Wq"], dtype=np.float32))
    Wk = np.ascontiguousarray(np.asarray(inputs["Wk"], dtype=np.float32))
    Wv = np.ascontiguousarray(np.asarray(inputs["Wv"], dtype=np.float32))
    bq = np.ascontiguousarray(
        np.asarray(inputs["bq"], dtype=np.float32).reshape(_K, 1))
    bk = np.ascontiguousarray(
        np.asarray(inputs["bk"], dtype=np.float32).reshape(_K, 1))
    bv = np.ascontiguousarray(
        np.asarray(inputs["bv"], dtype=np.float32).reshape(1, _V))

    nc = _get_nc()
    in_maps = []
    for core in range(8):
        b, g = divmod(core, 2)
        # this core's j columns: tiles {2k+g}, i.e. starts 256k + 128g
        cols = ((np.arange(_JT) * 256 + 128 * g)[:, None]
                + np.arange(128)[None, :]).ravel()
        in_maps.append({
            "xb": np.ascontiguousarray(x[b]),
            "xj": np.ascontiguousarray(x[b][:, cols]),
            "wq": Wq, "wk": Wk, "wv": Wv,
            "bq": bq, "bk": bk, "bv": bv,
            "mask": _masks(g),
        })

    trace = bool(_cache.get("trace"))
    res = run_bass_kernel_spmd(nc, in_maps, core_ids=list(range(8)),
                               trace=trace)
    _cache["last_result"] = res

    parts = [r["out"] for r in res.results]
    out = np.empty((_B, _C + _V, _T), np.float32)
    for b in range(_B):
        out[b, :_C] = x[b]
        out[b, _C:] = parts[2 * b] + parts[2 * b + 1]
    return out



# revision 2
# speedup vs baseline: 1.4217x; 1.4217x over previous
"""Trainium2 Bass kernel for nn_AttentionBlock (B=4, C=128, T=4096, K=64, V=128).

Sharding: 8 cores = 4 batches x 2 j-groups (data parallel over batch, plus a
split of the key/value axis j; the host sums the two partial read matrices).

Design notes (v2, restructured for ScalarE-bound overlap):
- The kernel is fundamentally bound by exp() on the Scalar engine
  (1 col/cycle @ 1.2 GHz, ~34.8K cols/core ~= 29 us). Everything else
  (PE ~20 us, DVE ~15 us, DMA ~5 us) is organized to hide under it.
- Host pre-casts x / weights to bf16: halves input DMA and removes all
  on-device cast traffic.
- Q^T / K^T are built with row-duplicated weights ([Wq|Wq]) so the two
  512-wide i-chunk QK^T matmuls (contraction K=64) can run CONCURRENTLY
  in the PE array via row tiling (rows 0-63 vs 64-127).
- Diagonal i-chunk of each j-tile is trimmed: the ACTIVATE (exp) spans only
  256 cols (even tiles) instead of 512; the dead tail of e is pre-zeroed.
- PSUM: tag "qk" 2 x [128,1536] (ping-pong: PE fills one group while
  ScalarE exps the other) + tag "ro" 2 x [128,512] (projections, V, readout).
- Output is DMA'd in bf16; host accumulates in f32.
"""

import numpy as np

_B, _C, _T = 4, 128, 4096
_K, _V = 64, 128
_JT = 16          # local 128-wide j tiles per core -> 2048 local j columns
_CH = 512         # i-chunk width (one PSUM bank in fp32)

_NEG = -1.0e30    # effective -inf for the causal mask (exp -> 0 exactly)
_LACT = (256, 512)  # activation span in the diagonal chunk, by tile parity

_cache = {}


def _build_nc():
    from contextlib import ExitStack

    import concourse.tile as tile
    from concourse import bacc, mybir
    from concourse.masks import make_identity

    f32 = mybir.dt.float32
    bf16 = mybir.dt.bfloat16
    AF = mybir.ActivationFunctionType

    nc = bacc.Bacc("TRN2", target_bir_lowering=False)

    xb_d = nc.dram_tensor("xb", [_C, _T], bf16, kind="ExternalInput")
    xj_d = nc.dram_tensor("xj", [_C, _JT * 128], bf16, kind="ExternalInput")
    wq_d = nc.dram_tensor("wq", [_C, 128], bf16, kind="ExternalInput")
    wk_d = nc.dram_tensor("wk", [_C, 128], bf16, kind="ExternalInput")
    wv_d = nc.dram_tensor("wv", [_C, _V], bf16, kind="ExternalInput")
    bq_d = nc.dram_tensor("bq", [128, 1], f32, kind="ExternalInput")
    bk_d = nc.dram_tensor("bk", [128, 1], f32, kind="ExternalInput")
    bv_d = nc.dram_tensor("bv", [1, _V], bf16, kind="ExternalInput")
    mk_d = nc.dram_tensor("mask", [2, 128, _CH], bf16, kind="ExternalInput")
    out_d = nc.dram_tensor("out", [_V, _T], bf16, kind="ExternalOutput")

    with tile.TileContext(nc) as tc, ExitStack() as ctx:
        singles = ctx.enter_context(tc.tile_pool(name="singles", bufs=1))
        work = ctx.enter_context(tc.tile_pool(name="work", bufs=2))
        small = ctx.enter_context(tc.tile_pool(name="small", bufs=4))
        psum = ctx.enter_context(tc.tile_pool(name="psum", bufs=1, space="PSUM"))

        # ---------------- input DMAs (already bf16) ----------------
        xb_bf = singles.tile([_C, _T], bf16)
        for c in range(4):
            nc.sync.dma_start(out=xb_bf[:, c * 1024:(c + 1) * 1024],
                              in_=xb_d[:, c * 1024:(c + 1) * 1024])
        xj_bf = singles.tile([_C, _JT * 128], bf16)
        for c in range(2):
            nc.sync.dma_start(out=xj_bf[:, c * 1024:(c + 1) * 1024],
                              in_=xj_d[:, c * 1024:(c + 1) * 1024])

        wq_bf = singles.tile([_C, 128], bf16)
        nc.sync.dma_start(out=wq_bf, in_=wq_d[:])
        wk_bf = singles.tile([_C, 128], bf16)
        nc.sync.dma_start(out=wk_bf, in_=wk_d[:])
        wv_bf = singles.tile([_C, _V], bf16)
        nc.sync.dma_start(out=wv_bf, in_=wv_d[:])
        bq_s = singles.tile([128, 1], f32)
        nc.sync.dma_start(out=bq_s, in_=bq_d[:])
        bk_s = singles.tile([128, 1], f32)
        nc.sync.dma_start(out=bk_s, in_=bk_d[:])
        bv_bf = singles.tile([1, _V], bf16)
        nc.sync.dma_start(out=bv_bf, in_=bv_d[:])
        mask_bf = singles.tile([128, 2, _CH], bf16)
        for m in range(2):
            nc.sync.dma_start(out=mask_bf[:, m, :], in_=mk_d[m])

        ones_bf = singles.tile([1, 128], bf16)
        nc.vector.memset(ones_bf, 1.0)
        id_bf = singles.tile([128, 128], bf16)
        make_identity(nc, id_bf[:])

        # ---------------- projections ----------------
        # qt[0:64] = Q^T, qt[64:128] = Q^T again (row-duplicated weights) so
        # QK^T matmuls can be row-tiled pairwise.
        qt_bf = singles.tile([128, _T], bf16)
        for g0, w in ((0, 3), (3, 3), (6, 2)):
            ps = psum.tile([128, 1536], f32, tag="qk", bufs=2, name="ps_qt")
            for j in range(w):
                c = g0 + j
                nc.tensor.matmul(ps[:, j * _CH:(j + 1) * _CH], wq_bf,
                                 xb_bf[:, c * _CH:(c + 1) * _CH],
                                 start=True, stop=True)
            nc.vector.tensor_scalar_add(
                qt_bf[:, g0 * _CH:(g0 + w) * _CH], ps[:, 0:w * _CH], bq_s[:])

        kt_bf = singles.tile([128, _JT * 128], bf16)
        # high chunk first: k=15 needs kt cols [1920:2048] as soon as possible
        ps = psum.tile([128, _CH], f32, tag="ro", bufs=2, name="ps_kt_hi")
        nc.tensor.matmul(ps[:], wk_bf, xj_bf[:, 3 * _CH:4 * _CH],
                         start=True, stop=True)
        nc.vector.tensor_scalar_add(kt_bf[:, 3 * _CH:4 * _CH], ps[:], bk_s[:])
        ps = psum.tile([128, 1536], f32, tag="qk", bufs=2, name="ps_kt_lo")
        for c in range(3):
            nc.tensor.matmul(ps[:, c * _CH:(c + 1) * _CH], wk_bf,
                             xj_bf[:, c * _CH:(c + 1) * _CH],
                             start=True, stop=True)
        nc.vector.tensor_scalar_add(kt_bf[:, 0:3 * _CH], ps[:, 0:3 * _CH],
                                    bk_s[:])

        # v[jl, v] = x_j^T Wv + bv, per 128-wide j-tile; 4 tiles per PSUM buf.
        v_f32 = singles.tile([128, _JT, _V], f32)
        for i in range(4):
            ps = psum.tile([128, _CH], f32, tag="ro", bufs=2, name="ps_v")
            for j in range(4):
                kk = 4 * i + j
                nc.tensor.matmul(ps[:, j * _V:(j + 1) * _V],
                                 xj_bf[:, kk * 128:(kk + 1) * 128],
                                 wv_bf, start=True, stop=False)
                nc.tensor.matmul(ps[:, j * _V:(j + 1) * _V], ones_bf, bv_bf,
                                 start=False, stop=True)
            nc.vector.tensor_copy(v_f32[:, 4 * i:4 * i + 4, :], ps[:])

        # ---------------- attention ----------------
        e_all = singles.tile([128, _JT, _T], bf16)
        vs_bf = singles.tile([128, _JT, _V], bf16)

        # pre-zero the dead tail of each even tile's diagonal chunk
        # (the exp ACTIVATE only covers the first _LACT[0] cols there)
        for k in range(0, _JT, 2):
            d = k // 2 + 1
            nc.vector.memset(
                e_all[:, k, (d - 1) * _CH + _LACT[0]:d * _CH], 0.0)

        def emit_read_chunk(c):
            ks = list(range(2 * c, _JT))
            ps = psum.tile([128, _CH], f32, tag="ro", bufs=2, name="ps_ro")
            for i, k in enumerate(ks):
                nc.tensor.matmul(ps[0:_V, :], vs_bf[:, k, :],
                                 e_all[:, k, c * _CH:(c + 1) * _CH],
                                 start=(i == 0), stop=(i == len(ks) - 1))
            ot = work.tile([_V, _CH], bf16, tag="osb")
            nc.vector.tensor_copy(ot, ps[0:_V, :])
            nc.sync.dma_start(out=out_d[:, c * _CH:(c + 1) * _CH], in_=ot)

        for k in range(_JT - 1, -1, -1):
            d = k // 2 + 1
            r = k % 2
            Ld = _LACT[r]
            accs = []
            for gs in range(0, d, 3):
                ge = min(gs + 3, d)
                ps = psum.tile([128, 1536], f32, tag="qk", bufs=2, name="ps_qk")
                for c in range(gs, ge):
                    off = (c - gs) * _CH
                    diag = (c == d - 1)
                    N = Ld if diag else _CH
                    h = 64 * (c % 2)
                    nc.tensor.matmul(
                        ps[:, off:off + N],
                        kt_bf[h:h + 64, k * 128:(k + 1) * 128],
                        qt_bf[h:h + 64, c * _CH:c * _CH + N],
                        start=True, stop=not diag)
                    if diag:
                        nc.tensor.matmul(ps[:, off:off + N], id_bf,
                                         mask_bf[:, r, 0:N],
                                         start=False, stop=True)
                fd = (ge - 1 - gs) * _CH + (Ld if ge == d else _CH)
                acc = small.tile([128, 1], f32, tag="acc", bufs=6)
                nc.scalar.activation(out=e_all[:, k, gs * _CH:gs * _CH + fd],
                                     in_=ps[:, 0:fd],
                                     func=AF.Exp, scale=0.125, accum_out=acc)
                accs.append(acc)
            s_t = accs[0]
            for a in accs[1:]:
                s_new = small.tile([128, 1], f32, tag="s", bufs=2)
                nc.vector.tensor_add(s_new, s_t, a)
                s_t = s_new
            rs = small.tile([128, 1], f32, tag="rs", bufs=2)
            nc.vector.reciprocal(rs, s_t)
            nc.vector.tensor_scalar_mul(vs_bf[:, k, :], v_f32[:, k, :], rs)

            if k % 2 == 0:
                emit_read_chunk(k // 2)

    nc.compile()
    return nc


def _get_nc():
    if "nc" not in _cache:
        _cache["nc"] = _build_nc()
    return _cache["nc"]


def _masks(g):
    """Additive causal-mask tiles (bf16) for a core in j-group g.

    Tile r (= local j-tile parity) masks the diagonal 512-wide i-chunk of
    every local j-tile with that parity: entry [p, ii] is live iff
    global_i <= global_j, i.e. ii <= (j0 - i0) + p with j0 - i0 = 128g + 256r.
    """
    import ml_dtypes

    m = np.zeros((2, 128, _CH), np.float32)
    p = np.arange(128)[:, None]
    ii = np.arange(_CH)[None, :]
    for parity in range(2):
        o = 128 * g + 256 * parity
        m[parity] = np.where(ii <= o + p, 0.0, _NEG)
    return m.astype(ml_dtypes.bfloat16)


def kernel(**inputs):
    import ml_dtypes

    from concourse.bass_utils import run_bass_kernel_spmd

    bf16 = ml_dtypes.bfloat16

    x = np.asarray(inputs["x"], dtype=np.float32)
    Wq = np.asarray(inputs["Wq"], dtype=np.float32)
    Wk = np.asarray(inputs["Wk"], dtype=np.float32)
    Wv = np.asarray(inputs["Wv"], dtype=np.float32)
    bq = np.asarray(inputs["bq"], dtype=np.float32).reshape(_K)
    bk = np.asarray(inputs["bk"], dtype=np.float32).reshape(_K)
    bv = np.asarray(inputs["bv"], dtype=np.float32).reshape(1, _V)

    xbf = x.astype(bf16)
    wq2 = np.ascontiguousarray(np.concatenate([Wq, Wq], axis=1)).astype(bf16)
    wk2 = np.ascontiguousarray(np.concatenate([Wk, Wk], axis=1)).astype(bf16)
    wv_b = np.ascontiguousarray(Wv).astype(bf16)
    bq2 = np.ascontiguousarray(np.concatenate([bq, bq]).reshape(128, 1))
    bk2 = np.ascontiguousarray(np.concatenate([bk, bk]).reshape(128, 1))
    bv_b = np.ascontiguousarray(bv).astype(bf16)

    nc = _get_nc()
    in_maps = []
    for core in range(8):
        b, g = divmod(core, 2)
        # this core's j columns: tiles {2k+g}, i.e. starts 256k + 128g
        cols = ((np.arange(_JT) * 256 + 128 * g)[:, None]
                + np.arange(128)[None, :]).ravel()
        in_maps.append({
            "xb": np.ascontiguousarray(xbf[b]),
            "xj": np.ascontiguousarray(xbf[b][:, cols]),
            "wq": wq2, "wk": wk2, "wv": wv_b,
            "bq": bq2, "bk": bk2, "bv": bv_b,
            "mask": _masks(g),
        })

    trace = bool(_cache.get("trace"))
    res = run_bass_kernel_spmd(nc, in_maps, core_ids=list(range(8)),
                               trace=trace)
    _cache["last_result"] = res

    parts = [r["out"] for r in res.results]
    out = np.empty((_B, _C + _V, _T), np.float32)
    for b in range(_B):
        out[b, :_C] = x[b]
        out[b, _C:] = (parts[2 * b].astype(np.float32)
                       + parts[2 * b + 1].astype(np.float32))
    return out
